# revision 33
# baseline (speedup 1.0000x reference)
"""MoE FFN (16 experts, top-2) Trainium2 Bass kernel.

Strategy (expert-parallel, per sharding hint):
  - Host: compute router logits/top-2/combine weights (float64 numpy; tiny
    fraction of FLOPs), dispatch tokens per expert with capacity padding.
  - Device (8 cores, SPMD): core c holds two experts — one from the 8
    highest-count experts (slot 0, capacity C0), one from the 8
    lowest-count (slot 1, capacity C1 <= C0) so padding waste is small.
    Per expert: y = silu(x @ W1 + b1) @ W2 over its gathered tokens.
    Activations stay feature-major ([D, tok] / [HID, tok]) so both
    matmuls contract on the partition dim with zero transposes on device.
    Matmuls run in float32r (full fp32 range, reduced-precision multiply,
    4x the fp32 rate; rel err ~1e-4 over K=1024).
  - Host: scatter-add combine (y * top2 weight + b2 term) back to [N, D].

All shapes hardcoded for the problem instance (D=1024, E=16, H=2048,
N=8192 tokens); capacities are computed at runtime from the routing.
"""

import numpy as np

DIM = 1024
NUM_EXPERTS = 16
TOP_K = 2
HID = 2048
AUX_COEFF = 1e-5
ROUTER_TEMP = 0.1
N_CORES = 8
P = 128
DKS = DIM // P    # 8  contraction chunks for fc1 / output chunks for fc2
HCS = HID // P    # 16 output chunks for fc1 / contraction chunks for fc2
EPC = NUM_EXPERTS // N_CORES  # experts per core = 2

# Results of the last device run (for test harness inspection).
LAST_RESULTS = None
LAST_NC = None
LAST_IN_MAPS = None


def _route(x, Wr, br):
    """Router in float64: top-2 indices, renormalized weights, aux loss."""
    xf = x.reshape(-1, DIM).astype(np.float64)
    logits = (xf @ Wr.astype(np.float64) + br.astype(np.float64)) / ROUTER_TEMP
    order = np.argsort(-logits, axis=1, kind="stable")
    top_i = order[:, :TOP_K]                       # [N, 2]
    top_v = np.take_along_axis(logits, top_i, axis=1)
    tw = np.exp(top_v - top_v[:, :1])
    tw = tw / tw.sum(axis=1, keepdims=True)        # [N, 2] renormalized
    pm = logits.max(axis=1, keepdims=True)
    pr = np.exp(logits - pm)
    pr /= pr.sum(axis=1, keepdims=True)
    aux = (pr.sum(axis=0) ** 2).sum() / NUM_EXPERTS * AUX_COEFF
    return top_i, tw, np.float32(aux)


def _capacity(cmax):
    """Smallest capacity >= cmax that splits into equal chunks in
    [256, 512] (float32r needs moving free dim >= 256 for full rate)."""
    n = max(1, -(-cmax // 512))
    while True:
        c = n * (-(-cmax // n))
        if c // n >= 256 or c < 256:
            return c, [(i * (c // n), c // n) for i in range(n)]
        n += 1


def _build_bass(caps):
    import concourse.mybir as mybir
    import concourse.tile as tile
    from concourse import bacc
    from contextlib import ExitStack

    f32 = mybir.dt.float32
    f32r = mybir.dt.float32r
    nc = bacc.Bacc("TRN2", debug=False, num_devices=N_CORES)

    xg, ys = [], []
    for s, C in enumerate(caps):
        xg.append(nc.dram_tensor(f"xg{s}", [DIM, C], f32r,
                                 kind="ExternalInput").ap())
        ys.append(nc.dram_tensor(f"y{s}", [DIM, C], f32,
                                 kind="ExternalOutput").ap())
    # w1t[e, hc, kin, dk, m]: W1[e].reshape(8,128,16,128).transpose(2,1,0,3)
    w1t = nc.dram_tensor("w1t", [EPC, HCS, P, DKS, P], f32r,
                         kind="ExternalInput").ap()
    # w2t[e, dc, kin, hk, m]: W2[e].reshape(16,128,8,128).transpose(2,1,0,3)
    w2t = nc.dram_tensor("w2t", [EPC, DKS, P, HCS, P], f32r,
                         kind="ExternalInput").ap()
    # b1g[e, p, hc] = b1[e, hc*128 + p]
    b1g = nc.dram_tensor("b1g", [EPC, P, HCS], f32, kind="ExternalInput").ap()

    cbig = max(caps)
    with tile.TileContext(nc) as tc, ExitStack() as ctx:
        xs_pool = ctx.enter_context(tc.tile_pool(name="xs", bufs=27))
        hs_pool = ctx.enter_context(tc.tile_pool(name="hs", bufs=HCS + 1))
        w1_pool = ctx.enter_context(tc.tile_pool(name="w1", bufs=3))
        w2_pool = ctx.enter_context(tc.tile_pool(name="w2", bufs=3))
        ys_pool = ctx.enter_context(tc.tile_pool(name="ys", bufs=4))
        b1_pool = ctx.enter_context(tc.tile_pool(name="b1", bufs=2))
        ps_pool = ctx.enter_context(tc.tile_pool(name="ps", bufs=8, space="PSUM"))

        for e, C in enumerate(caps):
            tts = _capacity(C)[1]

            # x subtiles split by token chunk, DMAs alternating across the
            # ACT queue and SP, in the order the first
            # matmuls consume them
            xs = {}
            for dk in range(DKS):
                for i, (t0, tsz) in enumerate(tts):
                    xt = xs_pool.tile([P, 512], f32r, tag="xs",
                                      name=f"xt_{e}_{i}_{dk}")
                    nc.sync.dma_start(
                        xt[:, :tsz], xg[e][dk * P:(dk + 1) * P, t0:t0 + tsz])
                    xs[dk, i] = xt

            # fc1 + silu: hs[hc] = silu(W1_chunk.T @ x + b1)
            # Startup: the first two hc columns run dk-outer together so
            # the PE has 2x the work per arriving x chunk during the fill.
            JH = 2
            hs = []
            w1s_j = []
            for g in range(JH):
                w1s = w1_pool.tile([P, DKS, P], f32r, tag="w1",
                                   name=f"w1j_{e}_{g}")
                for dk in range(DKS):
                    nc.gpsimd.dma_start(w1s[:, dk, :], w1t[e, g, :, dk, :])
                w1s_j.append(w1s)
            b1s = b1_pool.tile([P, HCS], f32, tag="b1")
            nc.gpsimd.dma_start(b1s[:], b1g[e])
            ps_j = [[ps_pool.tile([P, 512], f32, tag="ps",
                                  name=f"psj_{e}_{g}_{i}")
                     for i in range(len(tts))] for g in range(JH)]
            for dk in range(DKS):
                for g in range(JH):
                    for i, (t0, tsz) in enumerate(tts):
                        nc.tensor.matmul(
                            ps_j[g][i][:, :tsz],
                            w1s_j[g][:, dk, :],
                            xs[dk, i][:, :tsz],
                            start=(dk == 0),
                            stop=(dk == DKS - 1),
                        )
            for g in range(JH):
                ht = hs_pool.tile([P, cbig], f32r, tag="hs",
                                  name=f"htj_{e}_{g}")
                for i, (t0, tsz) in enumerate(tts):
                    nc.scalar.activation(
                        ht[:, t0:t0 + tsz],
                        ps_j[g][i][:, :tsz],
                        mybir.ActivationFunctionType.Silu,
                        bias=b1s[:, g:g + 1],
                    )
                hs.append(ht)
            for hc in range(JH, HCS):

                w1s = w1_pool.tile([P, DKS, P], f32r, tag="w1")
                nc.gpsimd.dma_start(w1s[:], w1t[e, hc])
                ht = hs_pool.tile([P, cbig], f32r, tag="hs")
                ps = [ps_pool.tile([P, 512], f32, tag="ps",
                                   name=f"ps_{e}_{hc}_{i}")
                      for i in range(len(tts))]
                for dk in range(DKS):
                    for i, (t0, tsz) in enumerate(tts):
                        nc.tensor.matmul(
                            ps[i][:, :tsz],
                            w1s[:, dk, :],
                            xs[dk, i][:, :tsz],
                            start=(dk == 0),
                            stop=(dk == DKS - 1),
                        )
                for i, (t0, tsz) in enumerate(tts):
                    nc.scalar.activation(
                        ht[:, t0:t0 + tsz],
                        ps[i][:, :tsz],
                        mybir.ActivationFunctionType.Silu,
                        bias=b1s[:, hc:hc + 1],
                    )
                hs.append(ht)

            # fc2: y[dc] = W2_chunk.T @ h; DMA straight from PSUM to DRAM
            for dc in range(DKS):
                w2s = w2_pool.tile([P, HCS, P], f32r, tag="w2")
                nc.gpsimd.dma_start(w2s[:], w2t[e, dc])
                ps = [ps_pool.tile([P, 512], f32, tag="ps",
                                   name=f"ps2_{e}_{dc}_{i}")
                      for i in range(len(tts))]
                for hk in range(HCS):
                    for i, (t0, tsz) in enumerate(tts):
                        nc.tensor.matmul(
                            ps[i][:, :tsz],
                            w2s[:, hk, :],
                            hs[hk][:, t0:t0 + tsz],
                            start=(hk == 0),
                            stop=(hk == HCS - 1),
                        )
                for i, (t0, tsz) in enumerate(tts):
                    yt = ys_pool.tile([P, 512], f32, tag="ys",
                                      name=f"yt_{e}_{dc}_{i}")
                    nc.scalar.copy(yt[:, :tsz], ps[i][:, :tsz])
                    nc.sync.dma_start(
                        ys[e][dc * P:(dc + 1) * P, t0:t0 + tsz],
                        yt[:, :tsz])

    nc.compile()
    return nc


def _prepare(inputs):
    """Host-side routing + dispatch. Returns (nc, in_maps, combine_ctx)."""
    x = np.ascontiguousarray(np.asarray(inputs["x"], dtype=np.float32))
    Wr = np.asarray(inputs["Wr"], dtype=np.float32)
    br = np.asarray(inputs["br"], dtype=np.float32)
    W1 = np.asarray(inputs["W1"], dtype=np.float32)
    b1 = np.asarray(inputs["b1"], dtype=np.float32)
    W2 = np.asarray(inputs["W2"], dtype=np.float32)
    b2 = np.asarray(inputs["b2"], dtype=np.float32)

    B, S, _ = x.shape
    N = B * S
    xf = x.reshape(N, DIM)

    top_i, tw, aux = _route(x, Wr, br)

    idx = [np.nonzero(top_i == e)[0] for e in range(NUM_EXPERTS)]
    pos = [np.argmax(top_i[idx[e]] == e, axis=1) for e in range(NUM_EXPERTS)]
    wts = [tw[idx[e], pos[e]].astype(np.float32) for e in range(NUM_EXPERTS)]
    counts = np.array([len(i) for i in idx])

    # slot 0 <- 8 highest-count experts, slot 1 <- 8 lowest-count
    order = np.argsort(-counts, kind="stable")
    big, small = order[:N_CORES], order[N_CORES:]
    C0, _ = _capacity(int(counts[big].max()))
    C1, _ = _capacity(int(counts[small].max()))
    caps = [C0, C1]
    # expert_of[core][slot]
    expert_of = [[int(big[c]), int(small[c])] for c in range(N_CORES)]

    nc = _build_bass(caps)

    in_maps = []
    for c in range(N_CORES):
        es = expert_of[c]
        m = {}
        for s, e in enumerate(es):
            xgc = np.zeros((DIM, caps[s]), np.float32)
            xgc[:, :counts[e]] = xf[idx[e]].T
            m[f"xg{s}"] = xgc
        m["w1t"] = np.ascontiguousarray(
            W1[es].reshape(EPC, DKS, P, HCS, P).transpose(0, 3, 2, 1, 4))
        m["w2t"] = np.ascontiguousarray(
            W2[es].reshape(EPC, HCS, P, DKS, P).transpose(0, 3, 2, 1, 4))
        m["b1g"] = np.ascontiguousarray(
            b1[es].reshape(EPC, HCS, P).transpose(0, 2, 1))
        in_maps.append(m)

    ctx = dict(B=B, S=S, N=N, idx=idx, wts=wts, counts=counts,
               expert_of=expert_of, b2=b2, aux=aux)
    return nc, in_maps, ctx


def _combine(results, ctx):
    N, B, S = ctx["N"], ctx["B"], ctx["S"]
    out = np.zeros((N, DIM), np.float32)
    slot_of = {}
    for c, es in enumerate(ctx["expert_of"]):
        for s, e in enumerate(es):
            slot_of[e] = (c, s)
    for e in range(NUM_EXPERTS):
        c, s = slot_of[e]
        ye = results[c][f"y{s}"]            # [DIM, C]
        cnt = ctx["counts"][e]
        contrib = ctx["wts"][e][:, None] * ye[:, :cnt].T
        if ctx["b2"][e].any():
            contrib = contrib + ctx["wts"][e][:, None] * ctx["b2"][e][None, :]
        out[ctx["idx"][e]] += contrib
    return out.reshape(B, S, DIM), ctx["aux"]


def kernel(**inputs):
    global LAST_RESULTS, LAST_NC, LAST_IN_MAPS
    from concourse.bass_utils import run_bass_kernel_spmd

    nc, in_maps, ctx = _prepare(inputs)
    LAST_NC, LAST_IN_MAPS = nc, in_maps
    res = run_bass_kernel_spmd(nc, in_maps, core_ids=list(range(N_CORES)))
    LAST_RESULTS = res
    return _combine(res.results, ctx)


# revision 40
# speedup vs baseline: 1.0101x; 1.0101x over previous
"""MoE FFN (16 experts, top-2) Trainium2 Bass kernel.

Strategy (expert-parallel, per sharding hint):
  - Host: compute router logits/top-2/combine weights (float64 numpy; tiny
    fraction of FLOPs), dispatch tokens per expert with capacity padding.
  - Device (8 cores, SPMD): core c holds two experts — one from the 8
    highest-count experts (slot 0, capacity C0), one from the 8
    lowest-count (slot 1, capacity C1 <= C0) so padding waste is small.
    Per expert: y = silu(x @ W1 + b1) @ W2 over its gathered tokens.
    Activations stay feature-major ([D, tok] / [HID, tok]) so both
    matmuls contract on the partition dim with zero transposes on device.
    Matmuls run in float32r (full fp32 range, reduced-precision multiply,
    4x the fp32 rate; rel err ~1e-4 over K=1024).
  - Host: scatter-add combine (y * top2 weight + b2 term) back to [N, D].

All shapes hardcoded for the problem instance (D=1024, E=16, H=2048,
N=8192 tokens); capacities are computed at runtime from the routing.
"""

import numpy as np

DIM = 1024
NUM_EXPERTS = 16
TOP_K = 2
HID = 2048
AUX_COEFF = 1e-5
ROUTER_TEMP = 0.1
N_CORES = 8
P = 128
DKS = DIM // P    # 8  contraction chunks for fc1 / output chunks for fc2
HCS = HID // P    # 16 output chunks for fc1 / contraction chunks for fc2
EPC = NUM_EXPERTS // N_CORES  # experts per core = 2

# Results of the last device run (for test harness inspection).
LAST_RESULTS = None
LAST_NC = None
LAST_IN_MAPS = None


def _route(x, Wr, br):
    """Router in float64: top-2 indices, renormalized weights, aux loss."""
    xf = x.reshape(-1, DIM).astype(np.float64)
    logits = (xf @ Wr.astype(np.float64) + br.astype(np.float64)) / ROUTER_TEMP
    order = np.argsort(-logits, axis=1, kind="stable")
    top_i = order[:, :TOP_K]                       # [N, 2]
    top_v = np.take_along_axis(logits, top_i, axis=1)
    tw = np.exp(top_v - top_v[:, :1])
    tw = tw / tw.sum(axis=1, keepdims=True)        # [N, 2] renormalized
    pm = logits.max(axis=1, keepdims=True)
    pr = np.exp(logits - pm)
    pr /= pr.sum(axis=1, keepdims=True)
    aux = (pr.sum(axis=0) ** 2).sum() / NUM_EXPERTS * AUX_COEFF
    return top_i, tw, np.float32(aux)


def _capacity(cmax):
    """Smallest capacity >= cmax that splits into equal chunks in
    [256, 512] (float32r needs moving free dim >= 256 for full rate)."""
    n = max(1, -(-cmax // 512))
    while True:
        c = n * (-(-cmax // n))
        if c // n >= 256 or c < 256:
            return c, [(i * (c // n), c // n) for i in range(n)]
        n += 1


def _build_bass(caps):
    import concourse.mybir as mybir
    import concourse.tile as tile
    from concourse import bacc
    from contextlib import ExitStack

    f32 = mybir.dt.float32
    f32r = mybir.dt.float32r
    nc = bacc.Bacc("TRN2", debug=False, num_devices=N_CORES)

    xg, ys = [], []
    for s, C in enumerate(caps):
        xg.append(nc.dram_tensor(f"xg{s}", [DIM, C], f32r,
                                 kind="ExternalInput").ap())
        ys.append(nc.dram_tensor(f"y{s}", [DIM, C], f32,
                                 kind="ExternalOutput").ap())
    # w1t[e, hc, kin, dk, m]: W1[e].reshape(8,128,16,128).transpose(2,1,0,3)
    w1t = nc.dram_tensor("w1t", [EPC, HCS, P, DKS, P], f32r,
                         kind="ExternalInput").ap()
    # w2t[e, dc, kin, hk, m]: W2[e].reshape(16,128,8,128).transpose(2,1,0,3)
    w2t = nc.dram_tensor("w2t", [EPC, DKS, P, HCS, P], f32r,
                         kind="ExternalInput").ap()
    # b1g[e, p, hc] = b1[e, hc*128 + p]
    b1g = nc.dram_tensor("b1g", [EPC, P, HCS], f32, kind="ExternalInput").ap()

    cbig = max(caps)
    with tile.TileContext(nc) as tc, ExitStack() as ctx:
        xs_pool = ctx.enter_context(tc.tile_pool(name="xs", bufs=27))
        hs_pool = ctx.enter_context(tc.tile_pool(name="hs", bufs=HCS + 1))
        w1_pool = ctx.enter_context(tc.tile_pool(name="w1", bufs=4))
        w2_pool = ctx.enter_context(tc.tile_pool(name="w2", bufs=3))
        ys_pool = ctx.enter_context(tc.tile_pool(name="ys", bufs=4))
        b1_pool = ctx.enter_context(tc.tile_pool(name="b1", bufs=2))
        ps_pool = ctx.enter_context(tc.tile_pool(name="ps", bufs=8, space="PSUM"))

        for e, C in enumerate(caps):
            tts = _capacity(C)[1]

            # x subtiles split by token chunk, DMAs alternating across the
            # ACT queue and SP, in the order the first
            # matmuls consume them
            xs = {}
            for dk in range(DKS):
                for i, (t0, tsz) in enumerate(tts):
                    xt = xs_pool.tile([P, 512], f32r, tag="xs",
                                      name=f"xt_{e}_{i}_{dk}")
                    nc.sync.dma_start(
                        xt[:, :tsz], xg[e][dk * P:(dk + 1) * P, t0:t0 + tsz])
                    xs[dk, i] = xt

            # fc1 + silu: hs[hc] = silu(W1_chunk.T @ x + b1)
            # Startup: the first two hc columns run dk-outer together so
            # the PE has 2x the work per arriving x chunk during the fill.
            JH = 2
            hs = []
            w1s_j = []
            for g in range(JH):
                w1s = w1_pool.tile([P, DKS, P], f32r, tag="w1",
                                   name=f"w1j_{e}_{g}")
                for dk in range(DKS):
                    nc.gpsimd.dma_start(w1s[:, dk, :], w1t[e, g, :, dk, :])
                w1s_j.append(w1s)
            b1s = b1_pool.tile([P, HCS], f32, tag="b1")
            nc.gpsimd.dma_start(b1s[:], b1g[e])
            # partial third column: hc=JH's first token-chunk group rides
            # the joint phase in the 7th PSUM bank (+16% PE fill density)
            w1s_x = w1_pool.tile([P, DKS, P], f32r, tag="w1",
                                 name=f"w1x_{e}")
            nc.gpsimd.dma_start(w1s_x[:], w1t[e, JH])
            ht_x = hs_pool.tile([P, cbig], f32r, tag="hs",
                                name=f"htx_{e}")
            ps_x = ps_pool.tile([P, 512], f32, tag="ps", name=f"psx_{e}")
            tsz0 = tts[0][1]
            ps_j = [[ps_pool.tile([P, 512], f32, tag="ps",
                                  name=f"psj_{e}_{g}_{i}")
                     for i in range(len(tts))] for g in range(JH)]
            for dk in range(DKS):
                for g in range(JH):
                    for i, (t0, tsz) in enumerate(tts):
                        nc.tensor.matmul(
                            ps_j[g][i][:, :tsz],
                            w1s_j[g][:, dk, :],
                            xs[dk, i][:, :tsz],
                            start=(dk == 0),
                            stop=(dk == DKS - 1),
                        )
                nc.tensor.matmul(
                    ps_x[:, :tsz0],
                    w1s_x[:, dk, :],
                    xs[dk, 0][:, :tsz0],
                    start=(dk == 0),
                    stop=(dk == DKS - 1),
                )
            for g in range(JH):
                ht = hs_pool.tile([P, cbig], f32r, tag="hs",
                                  name=f"htj_{e}_{g}")
                for i, (t0, tsz) in enumerate(tts):
                    nc.scalar.activation(
                        ht[:, t0:t0 + tsz],
                        ps_j[g][i][:, :tsz],
                        mybir.ActivationFunctionType.Silu,
                        bias=b1s[:, g:g + 1],
                    )
                hs.append(ht)
            nc.scalar.activation(
                ht_x[:, :tsz0], ps_x[:, :tsz0],
                mybir.ActivationFunctionType.Silu,
                bias=b1s[:, JH:JH + 1],
            )
            for hc in range(JH, HCS):
                if hc == JH:
                    w1s, ht, i0n = w1s_x, ht_x, 1
                else:
                    w1s = w1_pool.tile([P, DKS, P], f32r, tag="w1")
                    nc.gpsimd.dma_start(w1s[:], w1t[e, hc])
                    ht = hs_pool.tile([P, cbig], f32r, tag="hs")
                    i0n = 0
                ps = [ps_pool.tile([P, 512], f32, tag="ps",
                                   name=f"ps_{e}_{hc}_{i}")
                      for i in range(i0n, len(tts))]
                for dk in range(DKS):
                    for j, (t0, tsz) in enumerate(tts[i0n:]):
                        nc.tensor.matmul(
                            ps[j][:, :tsz],
                            w1s[:, dk, :],
                            xs[dk, i0n + j][:, :tsz],
                            start=(dk == 0),
                            stop=(dk == DKS - 1),
                        )
                for j, (t0, tsz) in enumerate(tts[i0n:]):
                    nc.scalar.activation(
                        ht[:, t0:t0 + tsz],
                        ps[j][:, :tsz],
                        mybir.ActivationFunctionType.Silu,
                        bias=b1s[:, hc:hc + 1],
                    )
                hs.append(ht)

            # fc2: y[dc] = W2_chunk.T @ h; DMA straight from PSUM to DRAM
            for dc in range(DKS):
                w2s = w2_pool.tile([P, HCS, P], f32r, tag="w2")
                nc.gpsimd.dma_start(w2s[:], w2t[e, dc])
                ps = [ps_pool.tile([P, 512], f32, tag="ps",
                                   name=f"ps2_{e}_{dc}_{i}")
                      for i in range(len(tts))]
                for hk in range(HCS):
                    for i, (t0, tsz) in enumerate(tts):
                        nc.tensor.matmul(
                            ps[i][:, :tsz],
                            w2s[:, hk, :],
                            hs[hk][:, t0:t0 + tsz],
                            start=(hk == 0),
                            stop=(hk == HCS - 1),
                        )
                for i, (t0, tsz) in enumerate(tts):
                    yt = ys_pool.tile([P, 512], f32, tag="ys",
                                      name=f"yt_{e}_{dc}_{i}")
                    nc.scalar.copy(yt[:, :tsz], ps[i][:, :tsz])
                    nc.sync.dma_start(
                        ys[e][dc * P:(dc + 1) * P, t0:t0 + tsz],
                        yt[:, :tsz])

    nc.compile()
    return nc


def _prepare(inputs):
    """Host-side routing + dispatch. Returns (nc, in_maps, combine_ctx)."""
    x = np.ascontiguousarray(np.asarray(inputs["x"], dtype=np.float32))
    Wr = np.asarray(inputs["Wr"], dtype=np.float32)
    br = np.asarray(inputs["br"], dtype=np.float32)
    W1 = np.asarray(inputs["W1"], dtype=np.float32)
    b1 = np.asarray(inputs["b1"], dtype=np.float32)
    W2 = np.asarray(inputs["W2"], dtype=np.float32)
    b2 = np.asarray(inputs["b2"], dtype=np.float32)

    B, S, _ = x.shape
    N = B * S
    xf = x.reshape(N, DIM)

    top_i, tw, aux = _route(x, Wr, br)

    idx = [np.nonzero(top_i == e)[0] for e in range(NUM_EXPERTS)]
    pos = [np.argmax(top_i[idx[e]] == e, axis=1) for e in range(NUM_EXPERTS)]
    wts = [tw[idx[e], pos[e]].astype(np.float32) for e in range(NUM_EXPERTS)]
    counts = np.array([len(i) for i in idx])

    # slot 0 <- 8 highest-count experts, slot 1 <- 8 lowest-count
    order = np.argsort(-counts, kind="stable")
    big, small = order[:N_CORES], order[N_CORES:]
    C0, _ = _capacity(int(counts[big].max()))
    C1, _ = _capacity(int(counts[small].max()))
    caps = [C0, C1]
    # expert_of[core][slot]
    expert_of = [[int(big[c]), int(small[c])] for c in range(N_CORES)]

    nc = _build_bass(caps)

    in_maps = []
    for c in range(N_CORES):
        es = expert_of[c]
        m = {}
        for s, e in enumerate(es):
            xgc = np.zeros((DIM, caps[s]), np.float32)
            xgc[:, :counts[e]] = xf[idx[e]].T
            m[f"xg{s}"] = xgc
        m["w1t"] = np.ascontiguousarray(
            W1[es].reshape(EPC, DKS, P, HCS, P).transpose(0, 3, 2, 1, 4))
        m["w2t"] = np.ascontiguousarray(
            W2[es].reshape(EPC, HCS, P, DKS, P).transpose(0, 3, 2, 1, 4))
        m["b1g"] = np.ascontiguousarray(
            b1[es].reshape(EPC, HCS, P).transpose(0, 2, 1))
        in_maps.append(m)

    ctx = dict(B=B, S=S, N=N, idx=idx, wts=wts, counts=counts,
               expert_of=expert_of, b2=b2, aux=aux)
    return nc, in_maps, ctx


def _combine(results, ctx):
    N, B, S = ctx["N"], ctx["B"], ctx["S"]
    out = np.zeros((N, DIM), np.float32)
    slot_of = {}
    for c, es in enumerate(ctx["expert_of"]):
        for s, e in enumerate(es):
            slot_of[e] = (c, s)
    for e in range(NUM_EXPERTS):
        c, s = slot_of[e]
        ye = results[c][f"y{s}"]            # [DIM, C]
        cnt = ctx["counts"][e]
        contrib = ctx["wts"][e][:, None] * ye[:, :cnt].T
        if ctx["b2"][e].any():
            contrib = contrib + ctx["wts"][e][:, None] * ctx["b2"][e][None, :]
        out[ctx["idx"][e]] += contrib
    return out.reshape(B, S, DIM), ctx["aux"]


def kernel(**inputs):
    global LAST_RESULTS, LAST_NC, LAST_IN_MAPS
    from concourse.bass_utils import run_bass_kernel_spmd

    nc, in_maps, ctx = _prepare(inputs)
    LAST_NC, LAST_IN_MAPS = nc, in_maps
    res = run_bass_kernel_spmd(nc, in_maps, core_ids=list(range(N_CORES)))
    LAST_RESULTS = res
    return _combine(res.results, ctx)
